# revision 18
# baseline (speedup 1.0000x reference)
"""Causal self-attention Trainium2 kernel (B=4, T=2048, D=1024, H=16).

Sharding: 8 cores = 4 batches x 2 head-groups (8 heads each). Each core
computes its batch's qkv projection restricted to its 8 heads, causal
attention for those heads, and a partial out-projection over its 512 ctx
channels. Host sums the two partials per batch and adds b_out.

Per-core layout choices (all matmuls bf16 with fp32 PSUM accumulation):
  - xT [C, T]: channels on partitions (contraction dim for projections).
  - qkT: per head-pair p, a q-tile [128, T] (head A rows 0:64, head B rows
    64:128) and a k-tile [128, T]. Produced directly transposed by making
    W the stationary operand. The 1/sqrt(dk) scale is folded into Wq/bq.
  - scoresT[s, t] blocks [128, 512]: lhsT=kT (K=64 rows), rhs=qT. Heads A/B
    are row-packed (tile_position rows 0:64 / 64:128) and run concurrently.
    Diagonal blocks only compute the causally needed t-range.
  - causal mask: diagonal 128x128 squares get an extra K=128 identity
    matmul accumulating a {0, -30000} triangular pattern; exp() gives 0.
  - softmax: no max-subtraction (scores are within +-10 by construction),
    exp on ScalarE PSUM->SBUF bf16.
  - ctx: v stored naturally [s, d] with a ones column appended per head
    (v_ext [128, 8*65]); lhsT=v_ext (M=65) so PSUM row 64 accumulates the
    softmax denominator. Normalize = reciprocal_approx_fast + gpsimd
    partition_broadcast + DVE mul into the bf16 ctxT copy.
  - out projection: ctxT pair-tiles [128, T] are the stationary operand
    against W_outT; b_out is added on the host (once per batch).

The main loop is i-tile-outer (t blocks of 512) so qk/v projection work,
attention for all 4 pairs, and the out-projection interleave: the PE
stays dense (HAM stays at K=8/8) and ScalarE exp overlaps matmuls.
"""

import math

import numpy as np
import ml_dtypes

B, T, C = 4, 2048, 1024
H, DK = 16, 64
NCORES = 8
TS = 128  # s-tile (partition granularity)
TSL = 512  # t free-dim tile (one PSUM bank of fp32)
MASK_VAL = -30000.0
BF16 = ml_dtypes.bfloat16


def build_program(C_sz=C, T_sz=T, n_pairs=4, num_devices=1):
    import concourse.mybir as mybir
    from concourse import bacc
    from concourse.tile import TileContext

    dt = mybir.dt
    f32 = dt.float32
    bf16 = dt.bfloat16
    AF = mybir.ActivationFunctionType

    n_ct = C_sz // 128  # contraction tiles for projections
    n_qk = 2 * n_pairs  # qk o-tiles (128 channels each)
    VW = n_pairs * 2 * DK  # v channels (natural order)
    n_tt = T_sz // TS
    n_it = T_sz // TSL
    JPI = TSL // TS  # s-tiles per i-tile (4)
    OW = min(TSL, C_sz)  # output column tile width
    n_oh = C_sz // OW  # output column halves
    VEW = n_pairs * 2 * (DK + 1)  # v_ext width (65 per head)

    nc = bacc.Bacc(
        "TRN2",
        target_bir_lowering=False,
        debug=False,
        num_devices=num_devices,
    )

    xT_d = nc.dram_tensor("xT", [C_sz, T_sz], bf16, kind="ExternalInput").ap()
    wqk_d = nc.dram_tensor("wqkT", [C_sz, n_qk * 128], bf16, kind="ExternalInput").ap()
    wv_d = nc.dram_tensor("wvT", [C_sz, VW], bf16, kind="ExternalInput").ap()
    bqk_d = nc.dram_tensor("bqk", [128, n_qk], f32, kind="ExternalInput").ap()
    bv_d = nc.dram_tensor("bv", [1, VW], bf16, kind="ExternalInput").ap()
    wo_d = nc.dram_tensor("woT", [n_pairs * 128, C_sz], bf16, kind="ExternalInput").ap()
    mask_d = nc.dram_tensor("masksq", [128, TS], bf16, kind="ExternalInput").ap()
    id_d = nc.dram_tensor("ident", [128, 128], bf16, kind="ExternalInput").ap()
    out_d = nc.dram_tensor("out", [T_sz, C_sz], f32, kind="ExternalOutput").ap()

    with TileContext(nc) as tc:
        with (
            tc.tile_pool(name="const", bufs=1) as const_pool,
            tc.tile_pool(name="big", bufs=1) as big_pool,
            tc.tile_pool(name="attn", bufs=10) as attn_pool,
            tc.tile_pool(name="rinv", bufs=6) as rinv_pool,
            tc.tile_pool(name="rbc", bufs=6) as rbc_pool,
            tc.tile_pool(name="outsb", bufs=6) as outsb_pool,
            tc.tile_pool(name="sc", bufs=2, space="PSUM") as sc_ps,
            tc.tile_pool(name="mm", bufs=4, space="PSUM") as mm_ps,
        ):
            # ---- weight/activation loads (first compute inputs first) ----
            xT_sb = []
            wqk_sb = []
            wv_sb = []
            for ci in range(n_ct):
                t = big_pool.tile([128, T_sz], bf16, tag=f"xT{ci}", name=f"xT{ci}")
                nc.sync.dma_start(t[:], xT_d[ci * 128 : (ci + 1) * 128, :])
                xT_sb.append(t)
                t = big_pool.tile(
                    [128, n_qk * 128], bf16, tag=f"wqk{ci}", name=f"wqk{ci}"
                )
                nc.sync.dma_start(t[:], wqk_d[ci * 128 : (ci + 1) * 128, :])
                wqk_sb.append(t)
            bqk_sb = const_pool.tile([128, n_qk], f32, tag="bqk", name="bqk")
            nc.sync.dma_start(bqk_sb[:], bqk_d)
            for ci in range(n_ct):
                t = big_pool.tile([128, VW], bf16, tag=f"wv{ci}", name=f"wv{ci}")
                nc.sync.dma_start(t[:], wv_d[ci * 128 : (ci + 1) * 128, :])
                wv_sb.append(t)
            bv_sb = const_pool.tile([1, VW], bf16, tag="bv", name="bv")
            nc.sync.dma_start(bv_sb[:], bv_d)
            bv_bc = const_pool.tile([128, VW], bf16, tag="bv_bc", name="bv_bc")
            nc.gpsimd.partition_broadcast(bv_bc[:], bv_sb[:])
            ident_sb = const_pool.tile([128, 128], bf16, tag="ident", name="ident")
            nc.sync.dma_start(ident_sb[:], id_d)
            mask_sb = const_pool.tile([128, TS], bf16, tag="mask", name="mask")
            nc.sync.dma_start(mask_sb[:], mask_d)
            wo_sb = []
            for p in range(n_pairs):
                t = big_pool.tile([128, C_sz], bf16, tag=f"wo{p}", name=f"wo{p}")
                nc.sync.dma_start(t[:], wo_d[p * 128 : (p + 1) * 128, :])
                wo_sb.append(t)

            qkT_sb = [
                big_pool.tile([128, T_sz], bf16, tag=f"qkT{ot}", name=f"qkT{ot}")
                for ot in range(n_qk)
            ]
            vext_sb = [
                big_pool.tile([128, VEW], bf16, tag=f"vext{tt}", name=f"vext{tt}")
                for tt in range(n_tt)
            ]
            ctxT_sb = [
                big_pool.tile([128, T_sz], bf16, tag=f"ctxT{p}", name=f"ctxT{p}")
                for p in range(n_pairs)
            ]

            def qk_proj(ot, i):
                ps = mm_ps.tile([128, TSL], f32, tag="mm", name="mm")
                for ci in range(n_ct):
                    nc.tensor.matmul(
                        ps[:],
                        lhsT=wqk_sb[ci][:, ot * 128 : (ot + 1) * 128],
                        rhs=xT_sb[ci][:, i * TSL : (i + 1) * TSL],
                        start=(ci == 0),
                        stop=(ci == n_ct - 1),
                    )
                nc.scalar.activation(
                    qkT_sb[ot][:, i * TSL : (i + 1) * TSL],
                    ps[:],
                    AF.Identity,
                    bias=bqk_sb[:, ot : ot + 1],
                )

            def v_proj(tt):
                ps = mm_ps.tile([128, VW], f32, tag="mm", name="mm")
                for ci in range(n_ct):
                    nc.tensor.matmul(
                        ps[:],
                        lhsT=xT_sb[ci][:, tt * TS : (tt + 1) * TS],
                        rhs=wv_sb[ci][:],
                        start=(ci == 0),
                        stop=(ci == n_ct - 1),
                    )
                vx = vext_sb[tt]
                vx3 = vx[:].rearrange("p (h e) -> p h e", e=DK + 1)
                nc.gpsimd.memset(vx3[:, :, DK : DK + 1], 1.0)
                nc.vector.scalar_tensor_tensor(
                    vx3[:, :, 0:DK],
                    ps[:].rearrange("p (h e) -> p h e", e=DK),
                    1.0,
                    bv_bc[:].rearrange("p (h e) -> p h e", e=DK),
                    op0=mybir.AluOpType.mult,
                    op1=mybir.AluOpType.add,
                )

            def out_proj(tt, oh):
                ps = mm_ps.tile([128, OW], f32, tag="mm", name="mm")
                for p in range(n_pairs):
                    nc.tensor.matmul(
                        ps[:],
                        lhsT=ctxT_sb[p][:, tt * TS : (tt + 1) * TS],
                        rhs=wo_sb[p][:, oh * OW : (oh + 1) * OW],
                        start=(p == 0),
                        stop=(p == n_pairs - 1),
                    )
                ob = outsb_pool.tile([128, OW], f32, tag="outsb", name="outsb")
                nc.scalar.activation(ob[:], ps[:], AF.Identity)
                nc.sync.dma_start(
                    out_d[tt * TS : (tt + 1) * TS, oh * OW : (oh + 1) * OW],
                    ob[:],
                )

            def attn_pair(p, i):
                qt, kt = qkT_sb[2 * p], qkT_sb[2 * p + 1]
                nj = JPI * (i + 1)
                ctxA = mm_ps.tile([DK + 1, TSL], f32, tag="mm", name="mm")
                ctxB = mm_ps.tile([DK + 1, TSL], f32, tag="mm", name="mm")
                for j in range(nj):
                    diag = j >= JPI * i
                    pi = j - JPI * i if diag else 0
                    t0 = pi * TS  # first causally-live t column in this block
                    ps = sc_ps.tile([128, 2 * TSL], f32, tag="sc", name="sc")
                    nc.tensor.matmul(
                        ps[:, t0:TSL],
                        lhsT=kt[0:64, j * TS : (j + 1) * TS],
                        rhs=qt[0:64, i * TSL + t0 : (i + 1) * TSL],
                        start=True,
                        stop=not diag,
                        skip_group_check=True,
                    )
                    nc.tensor.matmul(
                        ps[:, TSL + t0 : 2 * TSL],
                        lhsT=kt[64:128, j * TS : (j + 1) * TS],
                        rhs=qt[64:128, i * TSL + t0 : (i + 1) * TSL],
                        start=True,
                        stop=not diag,
                        skip_group_check=True,
                    )
                    if diag:
                        nc.tensor.matmul(
                            ps[:, t0 : t0 + TS],
                            lhsT=ident_sb[:],
                            rhs=mask_sb[:],
                            start=False,
                            stop=True,
                            skip_group_check=True,
                        )
                        nc.tensor.matmul(
                            ps[:, TSL + t0 : TSL + t0 + TS],
                            lhsT=ident_sb[:],
                            rhs=mask_sb[:],
                            start=False,
                            stop=True,
                            skip_group_check=True,
                        )
                    a = attn_pool.tile([128, 2 * TSL], bf16, tag="attn", name="attn")
                    a3 = a[:].rearrange("p (c w) -> p c w", c=2)
                    ps3 = ps[:].rearrange("p (c w) -> p c w", c=2)
                    nc.scalar.activation(a3[:, :, t0:TSL], ps3[:, :, t0:TSL], AF.Exp)
                    nc.tensor.matmul(
                        ctxA[:, t0:TSL],
                        lhsT=vext_sb[j][:, (2 * p) * (DK + 1) : (2 * p + 1) * (DK + 1)],
                        rhs=a[:, t0:TSL],
                        start=(j == 0),
                        stop=(j == nj - 1),
                    )
                    nc.tensor.matmul(
                        ctxB[:, t0:TSL],
                        lhsT=vext_sb[j][
                            :, (2 * p + 1) * (DK + 1) : (2 * p + 2) * (DK + 1)
                        ],
                        rhs=a[:, TSL + t0 : 2 * TSL],
                        start=(j == 0),
                        stop=(j == nj - 1),
                    )
                isl = slice(i * TSL, (i + 1) * TSL)
                for cps, rows in ((ctxA, slice(0, 64)), (ctxB, slice(64, 128))):
                    # custom-DVE ops misread PSUM on hw: bounce rowsum via SBUF
                    rs = rinv_pool.tile([1, TSL], f32, tag="rsum", name="rsum")
                    nc.vector.tensor_copy(rs[:], cps[DK : DK + 1, :])
                    r = rinv_pool.tile([1, TSL], f32, tag="rinv", name="rinv")
                    nc.vector.reciprocal_approx_fast(r[:], rs[:])
                    rbc = rbc_pool.tile([DK, TSL], f32, tag="rbc", name="rbc")
                    nc.gpsimd.partition_broadcast(rbc[:], r[:])
                    nc.vector.tensor_mul(ctxT_sb[p][rows, isl], cps[0:DK, :], rbc[:])

            # ---- main i-outer loop ----
            # projections for i+1 are emitted between attn(i) and out(i) so
            # the PE has independent work while the last pair normalizes.
            for ot in range(n_qk):
                qk_proj(ot, 0)
            for tt in range(0, JPI):
                v_proj(tt)
            for i in range(n_it):
                for p in range(n_pairs):
                    attn_pair(p, i)
                if i + 1 < n_it:
                    for ot in range(n_qk):
                        qk_proj(ot, i + 1)
                    for tt in range(JPI * (i + 1), JPI * (i + 2)):
                        v_proj(tt)
                for tt in range(JPI * i, JPI * (i + 1)):
                    for oh in range(n_oh):
                        out_proj(tt, oh)

    nc.compile()
    return nc


def make_mask_square(ts=TS):
    """[128, ts] strict lower-triangular: cell (s, t) = MASK_VAL iff s > t."""
    s = np.arange(128)[:, None]
    t = np.arange(ts)[None, :]
    return np.where(s > t, MASK_VAL, 0.0).astype(np.float32)


def make_core_inputs(x_b, W_qkv, b_qkv, W_out, heads, C_sz=C, T_sz=T):
    """Build the per-core input map (numpy, host-side)."""
    n_pairs = len(heads) // 2
    n_qk = 2 * n_pairs
    VW = len(heads) * DK
    xT = np.ascontiguousarray(x_b.T).astype(BF16)
    wqk = np.empty((C_sz, n_qk * 128), np.float32)
    bqk = np.empty((128, n_qk), np.float32)
    wv = np.empty((C_sz, VW), np.float32)
    bv = np.empty((1, VW), np.float32)
    wo = np.empty((n_pairs * 128, C_sz), np.float32)
    for p in range(n_pairs):
        hA, hB = heads[2 * p], heads[2 * p + 1]
        # q tile (scaled by 1/sqrt(dk)=1/8), k tile
        for half, h in ((0, hA), (1, hB)):
            r0 = h * 3 * DK
            wqk[:, 2 * p * 128 + half * 64 : 2 * p * 128 + half * 64 + 64] = (
                W_qkv[r0 : r0 + DK].T / math.sqrt(DK)
            )
            bqk[half * 64 : half * 64 + 64, 2 * p] = b_qkv[r0 : r0 + DK] / math.sqrt(DK)
            wqk[:, (2 * p + 1) * 128 + half * 64 : (2 * p + 1) * 128 + half * 64 + 64] = (
                W_qkv[r0 + DK : r0 + 2 * DK].T
            )
            bqk[half * 64 : half * 64 + 64, 2 * p + 1] = b_qkv[r0 + DK : r0 + 2 * DK]
            wo[p * 128 + half * 64 : p * 128 + half * 64 + 64, :] = W_out[
                :, h * DK : (h + 1) * DK
            ].T
    for hh, h in enumerate(heads):
        r0 = h * 3 * DK + 2 * DK
        wv[:, hh * DK : (hh + 1) * DK] = W_qkv[r0 : r0 + DK].T
        bv[0, hh * DK : (hh + 1) * DK] = b_qkv[r0 : r0 + DK]
    return {
        "xT": xT,
        "wqkT": wqk.astype(BF16),
        "wvT": wv.astype(BF16),
        "bqk": bqk.astype(np.float32),
        "bv": bv.astype(BF16),
        "woT": wo.astype(BF16),
        "masksq": make_mask_square().astype(BF16),
        "ident": np.eye(128, dtype=np.float32).astype(BF16),
    }


_NC_CACHE = {}


def kernel(x, W_qkv, b_qkv, W_out, b_out, _trace=False):
    x = np.asarray(x, dtype=np.float32)
    W_qkv = np.asarray(W_qkv, dtype=np.float32)
    b_qkv = np.asarray(b_qkv, dtype=np.float32)
    W_out = np.asarray(W_out, dtype=np.float32)
    b_out = np.asarray(b_out, dtype=np.float32)

    from concourse.bass_utils import run_bass_kernel_spmd

    key = ("full", C, T, 4)
    if key not in _NC_CACHE:
        _NC_CACHE[key] = build_program(C, T, n_pairs=4, num_devices=1)
    nc = _NC_CACHE[key]

    in_maps = []
    for core in range(NCORES):
        b, hg = divmod(core, 2)
        heads = list(range(hg * 8, hg * 8 + 8))
        in_maps.append(make_core_inputs(x[b], W_qkv, b_qkv, W_out, heads))

    res = run_bass_kernel_spmd(nc, in_maps, list(range(NCORES)), trace=_trace)
    kernel._last_results = res

    out = np.broadcast_to(b_out, (B, T, C)).astype(np.float32).copy()
    for core in range(NCORES):
        b = core // 2
        out[b] += res.results[core]["out"]
    return out


# revision 19
# speedup vs baseline: 1.0290x; 1.0290x over previous
"""Causal self-attention Trainium2 kernel (B=4, T=2048, D=1024, H=16).

Sharding: 8 cores = 4 batches x 2 head-groups (8 heads each). Each core
computes its batch's qkv projection restricted to its 8 heads, causal
attention for those heads, and a partial out-projection over its 512 ctx
channels. Host sums the two partials per batch and adds b_out.

Per-core layout choices (all matmuls bf16 with fp32 PSUM accumulation):
  - xT [C, T]: channels on partitions (contraction dim for projections).
  - qkT: per head-pair p, a q-tile [128, T] (head A rows 0:64, head B rows
    64:128) and a k-tile [128, T]. Produced directly transposed by making
    W the stationary operand. The 1/sqrt(dk) scale is folded into Wq/bq.
  - scoresT[s, t] blocks [128, 512]: lhsT=kT (K=64 rows), rhs=qT. Heads A/B
    are row-packed (tile_position rows 0:64 / 64:128) and run concurrently.
    Diagonal blocks only compute the causally needed t-range.
  - causal mask: diagonal 128x128 squares get an extra K=128 identity
    matmul accumulating a {0, -30000} triangular pattern; exp() gives 0.
  - softmax: no max-subtraction (scores are within +-10 by construction),
    exp on ScalarE PSUM->SBUF bf16.
  - ctx: v stored naturally [s, d] with a ones column appended per head
    (v_ext [128, 8*65]); lhsT=v_ext (M=65) so PSUM row 64 accumulates the
    softmax denominator. Normalize = reciprocal_approx_fast + gpsimd
    partition_broadcast + DVE mul into the bf16 ctxT copy.
  - out projection: ctxT pair-tiles [128, T] are the stationary operand
    against W_outT; b_out is added on the host (once per batch).

The main loop is i-tile-outer (t blocks of 512) so qk/v projection work,
attention for all 4 pairs, and the out-projection interleave: the PE
stays dense (HAM stays at K=8/8) and ScalarE exp overlaps matmuls.
"""

import math

import numpy as np
import ml_dtypes

B, T, C = 4, 2048, 1024
H, DK = 16, 64
NCORES = 8
TS = 128  # s-tile (partition granularity)
TSL = 512  # t free-dim tile (one PSUM bank of fp32)
MASK_VAL = -30000.0
BF16 = ml_dtypes.bfloat16


def build_program(C_sz=C, T_sz=T, n_pairs=4, num_devices=1):
    import concourse.mybir as mybir
    from concourse import bacc
    from concourse.tile import TileContext

    dt = mybir.dt
    f32 = dt.float32
    bf16 = dt.bfloat16
    AF = mybir.ActivationFunctionType

    n_ct = C_sz // 128  # contraction tiles for projections
    n_qk = 2 * n_pairs  # qk o-tiles (128 channels each)
    VW = n_pairs * 2 * DK  # v channels (natural order)
    n_tt = T_sz // TS
    n_it = T_sz // TSL
    JPI = TSL // TS  # s-tiles per i-tile (4)
    OW = min(TSL, C_sz)  # output column tile width
    n_oh = C_sz // OW  # output column halves
    VEW = n_pairs * 2 * (DK + 1)  # v_ext width (65 per head)

    nc = bacc.Bacc(
        "TRN2",
        target_bir_lowering=False,
        debug=False,
        num_devices=num_devices,
    )

    xT_d = nc.dram_tensor("xT", [C_sz, T_sz], bf16, kind="ExternalInput").ap()
    wqk_d = nc.dram_tensor("wqkT", [C_sz, n_qk * 128], bf16, kind="ExternalInput").ap()
    wv_d = nc.dram_tensor("wvT", [C_sz, VW], bf16, kind="ExternalInput").ap()
    bqk_d = nc.dram_tensor("bqk", [128, n_qk], f32, kind="ExternalInput").ap()
    bv_d = nc.dram_tensor("bv", [1, VW], bf16, kind="ExternalInput").ap()
    wo_d = nc.dram_tensor("woT", [n_pairs * 128, C_sz], bf16, kind="ExternalInput").ap()
    mask_d = nc.dram_tensor("masksq", [128, TS], bf16, kind="ExternalInput").ap()
    id_d = nc.dram_tensor("ident", [128, 128], bf16, kind="ExternalInput").ap()
    out_d = nc.dram_tensor("out", [T_sz, C_sz], f32, kind="ExternalOutput").ap()

    with TileContext(nc) as tc:
        with (
            tc.tile_pool(name="const", bufs=1) as const_pool,
            tc.tile_pool(name="big", bufs=1) as big_pool,
            tc.tile_pool(name="attn", bufs=10) as attn_pool,
            tc.tile_pool(name="rinv", bufs=6) as rinv_pool,
            tc.tile_pool(name="rbc", bufs=6) as rbc_pool,
            tc.tile_pool(name="outsb", bufs=6) as outsb_pool,
            tc.tile_pool(name="sc", bufs=2, space="PSUM") as sc_ps,
            tc.tile_pool(name="mm", bufs=4, space="PSUM") as mm_ps,
        ):
            # ---- weight/activation loads (first compute inputs first) ----
            xT_sb = []
            wqk_sb = []
            wv_sb = []
            for ci in range(n_ct):
                t = big_pool.tile([128, T_sz], bf16, tag=f"xT{ci}", name=f"xT{ci}")
                nc.sync.dma_start(t[:], xT_d[ci * 128 : (ci + 1) * 128, :])
                xT_sb.append(t)
                t = big_pool.tile(
                    [128, n_qk * 128], bf16, tag=f"wqk{ci}", name=f"wqk{ci}"
                )
                nc.sync.dma_start(t[:], wqk_d[ci * 128 : (ci + 1) * 128, :])
                wqk_sb.append(t)
            bqk_sb = const_pool.tile([128, n_qk], f32, tag="bqk", name="bqk")
            nc.sync.dma_start(bqk_sb[:], bqk_d)
            for ci in range(n_ct):
                t = big_pool.tile([128, VW], bf16, tag=f"wv{ci}", name=f"wv{ci}")
                nc.sync.dma_start(t[:], wv_d[ci * 128 : (ci + 1) * 128, :])
                wv_sb.append(t)
            bv_sb = const_pool.tile([1, VW], bf16, tag="bv", name="bv")
            nc.sync.dma_start(bv_sb[:], bv_d)
            bv_bc = const_pool.tile([128, VW], bf16, tag="bv_bc", name="bv_bc")
            nc.gpsimd.partition_broadcast(bv_bc[:], bv_sb[:])
            ident_sb = const_pool.tile([128, 128], bf16, tag="ident", name="ident")
            nc.sync.dma_start(ident_sb[:], id_d)
            mask_sb = const_pool.tile([128, TS], bf16, tag="mask", name="mask")
            nc.sync.dma_start(mask_sb[:], mask_d)
            wo_sb = []
            for p in range(n_pairs):
                t = big_pool.tile([128, C_sz], bf16, tag=f"wo{p}", name=f"wo{p}")
                nc.sync.dma_start(t[:], wo_d[p * 128 : (p + 1) * 128, :])
                wo_sb.append(t)

            qkT_sb = [
                big_pool.tile([128, T_sz], bf16, tag=f"qkT{ot}", name=f"qkT{ot}")
                for ot in range(n_qk)
            ]
            vext_sb = [
                big_pool.tile([128, VEW], bf16, tag=f"vext{tt}", name=f"vext{tt}")
                for tt in range(n_tt)
            ]
            ctxT_sb = [
                big_pool.tile([128, T_sz], bf16, tag=f"ctxT{p}", name=f"ctxT{p}")
                for p in range(n_pairs)
            ]

            def qk_proj(ot, i):
                ps = mm_ps.tile([128, TSL], f32, tag="mm", name="mm")
                for ci in range(n_ct):
                    nc.tensor.matmul(
                        ps[:],
                        lhsT=wqk_sb[ci][:, ot * 128 : (ot + 1) * 128],
                        rhs=xT_sb[ci][:, i * TSL : (i + 1) * TSL],
                        start=(ci == 0),
                        stop=(ci == n_ct - 1),
                    )
                nc.scalar.activation(
                    qkT_sb[ot][:, i * TSL : (i + 1) * TSL],
                    ps[:],
                    AF.Identity,
                    bias=bqk_sb[:, ot : ot + 1],
                )

            def v_proj(tt):
                ps = mm_ps.tile([128, VW], f32, tag="mm", name="mm")
                for ci in range(n_ct):
                    nc.tensor.matmul(
                        ps[:],
                        lhsT=xT_sb[ci][:, tt * TS : (tt + 1) * TS],
                        rhs=wv_sb[ci][:],
                        start=(ci == 0),
                        stop=(ci == n_ct - 1),
                    )
                vx = vext_sb[tt]
                vx3 = vx[:].rearrange("p (h e) -> p h e", e=DK + 1)
                nc.gpsimd.memset(vx3[:, :, DK : DK + 1], 1.0)
                nc.vector.scalar_tensor_tensor(
                    vx3[:, :, 0:DK],
                    ps[:].rearrange("p (h e) -> p h e", e=DK),
                    1.0,
                    bv_bc[:].rearrange("p (h e) -> p h e", e=DK),
                    op0=mybir.AluOpType.mult,
                    op1=mybir.AluOpType.add,
                )

            def out_proj(tt, oh):
                ps = mm_ps.tile([128, OW], f32, tag="mm", name="mm")
                for p in range(n_pairs):
                    nc.tensor.matmul(
                        ps[:],
                        lhsT=ctxT_sb[p][:, tt * TS : (tt + 1) * TS],
                        rhs=wo_sb[p][:, oh * OW : (oh + 1) * OW],
                        start=(p == 0),
                        stop=(p == n_pairs - 1),
                    )
                ob = outsb_pool.tile([128, OW], f32, tag="outsb", name="outsb")
                nc.scalar.activation(ob[:], ps[:], AF.Identity)
                nc.sync.dma_start(
                    out_d[tt * TS : (tt + 1) * TS, oh * OW : (oh + 1) * OW],
                    ob[:],
                )

            def attn_pair(p, i):
                qt, kt = qkT_sb[2 * p], qkT_sb[2 * p + 1]
                nj = JPI * (i + 1)
                ctxA = mm_ps.tile([DK + 1, TSL], f32, tag="mm", name="mm")
                ctxB = mm_ps.tile([DK + 1, TSL], f32, tag="mm", name="mm")
                for j in range(nj):
                    diag = j >= JPI * i
                    pi = j - JPI * i if diag else 0
                    t0 = pi * TS  # first causally-live t column in this block
                    ps = sc_ps.tile([128, 2 * TSL], f32, tag="sc", name="sc")
                    nc.tensor.matmul(
                        ps[:, t0:TSL],
                        lhsT=kt[0:64, j * TS : (j + 1) * TS],
                        rhs=qt[0:64, i * TSL + t0 : (i + 1) * TSL],
                        start=True,
                        stop=not diag,
                        skip_group_check=True,
                    )
                    nc.tensor.matmul(
                        ps[:, TSL + t0 : 2 * TSL],
                        lhsT=kt[64:128, j * TS : (j + 1) * TS],
                        rhs=qt[64:128, i * TSL + t0 : (i + 1) * TSL],
                        start=True,
                        stop=not diag,
                        skip_group_check=True,
                    )
                    if diag:
                        nc.tensor.matmul(
                            ps[:, t0 : t0 + TS],
                            lhsT=ident_sb[:],
                            rhs=mask_sb[:],
                            start=False,
                            stop=True,
                            skip_group_check=True,
                        )
                        nc.tensor.matmul(
                            ps[:, TSL + t0 : TSL + t0 + TS],
                            lhsT=ident_sb[:],
                            rhs=mask_sb[:],
                            start=False,
                            stop=True,
                            skip_group_check=True,
                        )
                    a = attn_pool.tile([128, 2 * TSL], bf16, tag="attn", name="attn")
                    a3 = a[:].rearrange("p (c w) -> p c w", c=2)
                    ps3 = ps[:].rearrange("p (c w) -> p c w", c=2)
                    nc.scalar.activation(a3[:, :, t0:TSL], ps3[:, :, t0:TSL], AF.Exp)
                    nc.tensor.matmul(
                        ctxA[:, t0:TSL],
                        lhsT=vext_sb[j][:, (2 * p) * (DK + 1) : (2 * p + 1) * (DK + 1)],
                        rhs=a[:, t0:TSL],
                        start=(j == 0),
                        stop=(j == nj - 1),
                    )
                    nc.tensor.matmul(
                        ctxB[:, t0:TSL],
                        lhsT=vext_sb[j][
                            :, (2 * p + 1) * (DK + 1) : (2 * p + 2) * (DK + 1)
                        ],
                        rhs=a[:, TSL + t0 : 2 * TSL],
                        start=(j == 0),
                        stop=(j == nj - 1),
                    )
                isl = slice(i * TSL, (i + 1) * TSL)
                for cps, rows in ((ctxA, slice(0, 64)), (ctxB, slice(64, 128))):
                    # custom-DVE ops misread PSUM on hw: bounce rowsum via SBUF
                    rs = rinv_pool.tile([1, TSL], f32, tag="rsum", name="rsum")
                    nc.vector.tensor_copy(rs[:], cps[DK : DK + 1, :])
                    r = rinv_pool.tile([1, TSL], f32, tag="rinv", name="rinv")
                    nc.vector.reciprocal_approx_fast(r[:], rs[:])
                    rbc = rbc_pool.tile([DK, TSL], f32, tag="rbc", name="rbc")
                    nc.gpsimd.partition_broadcast(rbc[:], r[:])
                    nc.vector.tensor_mul(ctxT_sb[p][rows, isl], cps[0:DK, :], rbc[:])

            # ---- main i-outer loop ----
            # projections for i+1 are emitted between attn(i) and out(i) so
            # the PE has independent work while the last pair normalizes.
            for ot in range(n_qk):
                qk_proj(ot, 0)
            for tt in range(0, JPI):
                v_proj(tt)
            for i in range(n_it):
                # pair 0 of iteration i was already emitted at the end of
                # iteration i-1 (pulled ahead so ScalarE gets exp work during
                # the projection segment).
                for p in range(0 if i == 0 else 1, n_pairs):
                    attn_pair(p, i)
                if i + 1 < n_it:
                    qk_proj(0, i + 1)
                    qk_proj(1, i + 1)
                    for tt in range(JPI * (i + 1), JPI * (i + 2)):
                        v_proj(tt)
                    attn_pair(0, i + 1)
                    for ot in range(2, n_qk):
                        qk_proj(ot, i + 1)
                for tt in range(JPI * i, JPI * (i + 1)):
                    for oh in range(n_oh):
                        out_proj(tt, oh)

    nc.compile()
    return nc


def make_mask_square(ts=TS):
    """[128, ts] strict lower-triangular: cell (s, t) = MASK_VAL iff s > t."""
    s = np.arange(128)[:, None]
    t = np.arange(ts)[None, :]
    return np.where(s > t, MASK_VAL, 0.0).astype(np.float32)


def make_core_inputs(x_b, W_qkv, b_qkv, W_out, heads, C_sz=C, T_sz=T):
    """Build the per-core input map (numpy, host-side)."""
    n_pairs = len(heads) // 2
    n_qk = 2 * n_pairs
    VW = len(heads) * DK
    xT = np.ascontiguousarray(x_b.T).astype(BF16)
    wqk = np.empty((C_sz, n_qk * 128), np.float32)
    bqk = np.empty((128, n_qk), np.float32)
    wv = np.empty((C_sz, VW), np.float32)
    bv = np.empty((1, VW), np.float32)
    wo = np.empty((n_pairs * 128, C_sz), np.float32)
    for p in range(n_pairs):
        hA, hB = heads[2 * p], heads[2 * p + 1]
        # q tile (scaled by 1/sqrt(dk)=1/8), k tile
        for half, h in ((0, hA), (1, hB)):
            r0 = h * 3 * DK
            wqk[:, 2 * p * 128 + half * 64 : 2 * p * 128 + half * 64 + 64] = (
                W_qkv[r0 : r0 + DK].T / math.sqrt(DK)
            )
            bqk[half * 64 : half * 64 + 64, 2 * p] = b_qkv[r0 : r0 + DK] / math.sqrt(DK)
            wqk[:, (2 * p + 1) * 128 + half * 64 : (2 * p + 1) * 128 + half * 64 + 64] = (
                W_qkv[r0 + DK : r0 + 2 * DK].T
            )
            bqk[half * 64 : half * 64 + 64, 2 * p + 1] = b_qkv[r0 + DK : r0 + 2 * DK]
            wo[p * 128 + half * 64 : p * 128 + half * 64 + 64, :] = W_out[
                :, h * DK : (h + 1) * DK
            ].T
    for hh, h in enumerate(heads):
        r0 = h * 3 * DK + 2 * DK
        wv[:, hh * DK : (hh + 1) * DK] = W_qkv[r0 : r0 + DK].T
        bv[0, hh * DK : (hh + 1) * DK] = b_qkv[r0 : r0 + DK]
    return {
        "xT": xT,
        "wqkT": wqk.astype(BF16),
        "wvT": wv.astype(BF16),
        "bqk": bqk.astype(np.float32),
        "bv": bv.astype(BF16),
        "woT": wo.astype(BF16),
        "masksq": make_mask_square().astype(BF16),
        "ident": np.eye(128, dtype=np.float32).astype(BF16),
    }


_NC_CACHE = {}


def kernel(x, W_qkv, b_qkv, W_out, b_out, _trace=False):
    x = np.asarray(x, dtype=np.float32)
    W_qkv = np.asarray(W_qkv, dtype=np.float32)
    b_qkv = np.asarray(b_qkv, dtype=np.float32)
    W_out = np.asarray(W_out, dtype=np.float32)
    b_out = np.asarray(b_out, dtype=np.float32)

    from concourse.bass_utils import run_bass_kernel_spmd

    key = ("full", C, T, 4)
    if key not in _NC_CACHE:
        _NC_CACHE[key] = build_program(C, T, n_pairs=4, num_devices=1)
    nc = _NC_CACHE[key]

    in_maps = []
    for core in range(NCORES):
        b, hg = divmod(core, 2)
        heads = list(range(hg * 8, hg * 8 + 8))
        in_maps.append(make_core_inputs(x[b], W_qkv, b_qkv, W_out, heads))

    res = run_bass_kernel_spmd(nc, in_maps, list(range(NCORES)), trace=_trace)
    kernel._last_results = res

    out = np.broadcast_to(b_out, (B, T, C)).astype(np.float32).copy()
    for core in range(NCORES):
        b = core // 2
        out[b] += res.results[core]["out"]
    return out


# revision 20
# speedup vs baseline: 1.0532x; 1.0235x over previous
"""Causal self-attention Trainium2 kernel (B=4, T=2048, D=1024, H=16).

Sharding: 8 cores = 4 batches x 2 head-groups (8 heads each). Each core
computes its batch's qkv projection restricted to its 8 heads, causal
attention for those heads, and a partial out-projection over its 512 ctx
channels. Host sums the two partials per batch and adds b_out.

Per-core layout choices (all matmuls bf16 with fp32 PSUM accumulation):
  - xT [C, T]: channels on partitions (contraction dim for projections).
  - qkT: per head-pair p, a q-tile [128, T] (head A rows 0:64, head B rows
    64:128) and a k-tile [128, T]. Produced directly transposed by making
    W the stationary operand. The 1/sqrt(dk) scale is folded into Wq/bq.
  - scoresT[s, t] blocks [128, 512]: lhsT=kT (K=64 rows), rhs=qT. Heads A/B
    are row-packed (tile_position rows 0:64 / 64:128) and run concurrently.
    Diagonal blocks only compute the causally needed t-range.
  - causal mask: diagonal 128x128 squares get an extra K=128 identity
    matmul accumulating a {0, -30000} triangular pattern; exp() gives 0.
  - softmax: no max-subtraction (scores are within +-10 by construction),
    exp on ScalarE PSUM->SBUF bf16.
  - ctx: v stored naturally [s, d] with a ones column appended per head
    (v_ext [128, 8*65]); lhsT=v_ext (M=65) so PSUM row 64 accumulates the
    softmax denominator. Normalize = reciprocal_approx_fast + gpsimd
    partition_broadcast + DVE mul into the bf16 ctxT copy.
  - out projection: ctxT pair-tiles [128, T] are the stationary operand
    against W_outT; b_out is added on the host (once per batch).

The main loop is i-tile-outer (t blocks of 512) so qk/v projection work,
attention for all 4 pairs, and the out-projection interleave: the PE
stays dense (HAM stays at K=8/8) and ScalarE exp overlaps matmuls.
"""

import math

import numpy as np
import ml_dtypes

B, T, C = 4, 2048, 1024
H, DK = 16, 64
NCORES = 8
TS = 128  # s-tile (partition granularity)
TSL = 512  # t free-dim tile (one PSUM bank of fp32)
MASK_VAL = -30000.0
BF16 = ml_dtypes.bfloat16


def build_program(C_sz=C, T_sz=T, n_pairs=4, num_devices=1):
    import concourse.mybir as mybir
    from concourse import bacc
    from concourse.tile import TileContext

    dt = mybir.dt
    f32 = dt.float32
    bf16 = dt.bfloat16
    AF = mybir.ActivationFunctionType

    n_ct = C_sz // 128  # contraction tiles for projections
    n_qk = 2 * n_pairs  # qk o-tiles (128 channels each)
    VW = n_pairs * 2 * DK  # v channels (natural order)
    n_tt = T_sz // TS
    n_it = T_sz // TSL
    JPI = TSL // TS  # s-tiles per i-tile (4)
    OW = min(TSL, C_sz)  # output column tile width
    n_oh = C_sz // OW  # output column halves
    VEW = n_pairs * 2 * (DK + 1)  # v_ext width (65 per head)

    nc = bacc.Bacc(
        "TRN2",
        target_bir_lowering=False,
        debug=False,
        num_devices=num_devices,
    )

    xT_d = nc.dram_tensor("xT", [C_sz, T_sz], bf16, kind="ExternalInput").ap()
    wqk_d = nc.dram_tensor("wqkT", [C_sz, n_qk * 128], bf16, kind="ExternalInput").ap()
    wv_d = nc.dram_tensor("wvT", [C_sz, VW], bf16, kind="ExternalInput").ap()
    bqk_d = nc.dram_tensor("bqk", [128, n_qk], f32, kind="ExternalInput").ap()
    bv_d = nc.dram_tensor("bv", [1, VW], bf16, kind="ExternalInput").ap()
    wo_d = nc.dram_tensor("woT", [n_pairs * 128, C_sz], bf16, kind="ExternalInput").ap()
    mask_d = nc.dram_tensor("masksq", [128, TS], bf16, kind="ExternalInput").ap()
    id_d = nc.dram_tensor("ident", [128, 128], bf16, kind="ExternalInput").ap()
    out_d = nc.dram_tensor("out", [T_sz, C_sz], f32, kind="ExternalOutput").ap()

    with TileContext(nc) as tc:
        with (
            tc.tile_pool(name="const", bufs=1) as const_pool,
            tc.tile_pool(name="big", bufs=1) as big_pool,
            tc.tile_pool(name="attn", bufs=10) as attn_pool,
            tc.tile_pool(name="rinv", bufs=6) as rinv_pool,
            tc.tile_pool(name="rbc", bufs=6) as rbc_pool,
            tc.tile_pool(name="outsb", bufs=6) as outsb_pool,
            tc.tile_pool(name="sc", bufs=2, space="PSUM") as sc_ps,
            tc.tile_pool(name="mm", bufs=4, space="PSUM") as mm_ps,
        ):
            # ---- weight/activation loads (first compute inputs first) ----
            xT_sb = []
            wqk_sb = []
            wv_sb = []
            for ci in range(n_ct):
                t = big_pool.tile([128, T_sz], bf16, tag=f"xT{ci}", name=f"xT{ci}")
                nc.sync.dma_start(t[:], xT_d[ci * 128 : (ci + 1) * 128, :])
                xT_sb.append(t)
                t = big_pool.tile(
                    [128, n_qk * 128], bf16, tag=f"wqk{ci}", name=f"wqk{ci}"
                )
                nc.sync.dma_start(t[:], wqk_d[ci * 128 : (ci + 1) * 128, :])
                wqk_sb.append(t)
            bqk_sb = const_pool.tile([128, n_qk], f32, tag="bqk", name="bqk")
            nc.sync.dma_start(bqk_sb[:], bqk_d)
            for ci in range(n_ct):
                t = big_pool.tile([128, VW], bf16, tag=f"wv{ci}", name=f"wv{ci}")
                nc.sync.dma_start(t[:], wv_d[ci * 128 : (ci + 1) * 128, :])
                wv_sb.append(t)
            bv_sb = const_pool.tile([1, VW], bf16, tag="bv", name="bv")
            nc.sync.dma_start(bv_sb[:], bv_d)
            bv_bc = const_pool.tile([128, VW], bf16, tag="bv_bc", name="bv_bc")
            nc.gpsimd.partition_broadcast(bv_bc[:], bv_sb[:])
            ident_sb = const_pool.tile([128, 128], bf16, tag="ident", name="ident")
            nc.sync.dma_start(ident_sb[:], id_d)
            mask_sb = const_pool.tile([128, TS], bf16, tag="mask", name="mask")
            nc.sync.dma_start(mask_sb[:], mask_d)
            wo_sb = []
            for p in range(n_pairs):
                t = big_pool.tile([128, C_sz], bf16, tag=f"wo{p}", name=f"wo{p}")
                nc.sync.dma_start(t[:], wo_d[p * 128 : (p + 1) * 128, :])
                wo_sb.append(t)

            qkT_sb = [
                big_pool.tile([128, T_sz], bf16, tag=f"qkT{ot}", name=f"qkT{ot}")
                for ot in range(n_qk)
            ]
            vext_sb = [
                big_pool.tile([128, VEW], bf16, tag=f"vext{tt}", name=f"vext{tt}")
                for tt in range(n_tt)
            ]
            ctxT_sb = [
                big_pool.tile([128, T_sz], bf16, tag=f"ctxT{p}", name=f"ctxT{p}")
                for p in range(n_pairs)
            ]

            def qk_proj(ot, i):
                ps = mm_ps.tile([128, TSL], f32, tag="mm", name="mm")
                for ci in range(n_ct):
                    nc.tensor.matmul(
                        ps[:],
                        lhsT=wqk_sb[ci][:, ot * 128 : (ot + 1) * 128],
                        rhs=xT_sb[ci][:, i * TSL : (i + 1) * TSL],
                        start=(ci == 0),
                        stop=(ci == n_ct - 1),
                    )
                nc.scalar.activation(
                    qkT_sb[ot][:, i * TSL : (i + 1) * TSL],
                    ps[:],
                    AF.Identity,
                    bias=bqk_sb[:, ot : ot + 1],
                )

            def v_proj(tt):
                ps = mm_ps.tile([128, VW], f32, tag="mm", name="mm")
                for ci in range(n_ct):
                    nc.tensor.matmul(
                        ps[:],
                        lhsT=xT_sb[ci][:, tt * TS : (tt + 1) * TS],
                        rhs=wv_sb[ci][:],
                        start=(ci == 0),
                        stop=(ci == n_ct - 1),
                    )
                vx = vext_sb[tt]
                vx3 = vx[:].rearrange("p (h e) -> p h e", e=DK + 1)
                nc.gpsimd.memset(vx3[:, :, DK : DK + 1], 1.0)
                nc.vector.scalar_tensor_tensor(
                    vx3[:, :, 0:DK],
                    ps[:].rearrange("p (h e) -> p h e", e=DK),
                    1.0,
                    bv_bc[:].rearrange("p (h e) -> p h e", e=DK),
                    op0=mybir.AluOpType.mult,
                    op1=mybir.AluOpType.add,
                )

            def out_proj(tt, oh):
                ps = mm_ps.tile([128, OW], f32, tag="mm", name="mm")
                for p in range(n_pairs):
                    nc.tensor.matmul(
                        ps[:],
                        lhsT=ctxT_sb[p][:, tt * TS : (tt + 1) * TS],
                        rhs=wo_sb[p][:, oh * OW : (oh + 1) * OW],
                        start=(p == 0),
                        stop=(p == n_pairs - 1),
                    )
                ob = outsb_pool.tile([128, OW], f32, tag="outsb", name="outsb")
                nc.scalar.activation(ob[:], ps[:], AF.Identity)
                nc.sync.dma_start(
                    out_d[tt * TS : (tt + 1) * TS, oh * OW : (oh + 1) * OW],
                    ob[:],
                )

            def attn_pair(p, i):
                qt, kt = qkT_sb[2 * p], qkT_sb[2 * p + 1]
                nj = JPI * (i + 1)
                ctxA = mm_ps.tile([DK + 1, TSL], f32, tag="mm", name="mm")
                ctxB = mm_ps.tile([DK + 1, TSL], f32, tag="mm", name="mm")
                for j in range(nj):
                    diag = j >= JPI * i
                    pi = j - JPI * i if diag else 0
                    t0 = pi * TS  # first causally-live t column in this block
                    ps = sc_ps.tile([128, 2 * TSL], f32, tag="sc", name="sc")
                    nc.tensor.matmul(
                        ps[:, t0:TSL],
                        lhsT=kt[0:64, j * TS : (j + 1) * TS],
                        rhs=qt[0:64, i * TSL + t0 : (i + 1) * TSL],
                        start=True,
                        stop=not diag,
                        skip_group_check=True,
                    )
                    nc.tensor.matmul(
                        ps[:, TSL + t0 : 2 * TSL],
                        lhsT=kt[64:128, j * TS : (j + 1) * TS],
                        rhs=qt[64:128, i * TSL + t0 : (i + 1) * TSL],
                        start=True,
                        stop=not diag,
                        skip_group_check=True,
                    )
                    if diag:
                        nc.tensor.matmul(
                            ps[:, t0 : t0 + TS],
                            lhsT=ident_sb[:],
                            rhs=mask_sb[:],
                            start=False,
                            stop=True,
                            skip_group_check=True,
                        )
                        nc.tensor.matmul(
                            ps[:, TSL + t0 : TSL + t0 + TS],
                            lhsT=ident_sb[:],
                            rhs=mask_sb[:],
                            start=False,
                            stop=True,
                            skip_group_check=True,
                        )
                    a = attn_pool.tile([128, 2 * TSL], bf16, tag="attn", name="attn")
                    a3 = a[:].rearrange("p (c w) -> p c w", c=2)
                    ps3 = ps[:].rearrange("p (c w) -> p c w", c=2)
                    nc.scalar.activation(a3[:, :, t0:TSL], ps3[:, :, t0:TSL], AF.Exp)
                    nc.tensor.matmul(
                        ctxA[:, t0:TSL],
                        lhsT=vext_sb[j][:, (2 * p) * (DK + 1) : (2 * p + 1) * (DK + 1)],
                        rhs=a[:, t0:TSL],
                        start=(j == 0),
                        stop=(j == nj - 1),
                    )
                    nc.tensor.matmul(
                        ctxB[:, t0:TSL],
                        lhsT=vext_sb[j][
                            :, (2 * p + 1) * (DK + 1) : (2 * p + 2) * (DK + 1)
                        ],
                        rhs=a[:, TSL + t0 : 2 * TSL],
                        start=(j == 0),
                        stop=(j == nj - 1),
                    )
                isl = slice(i * TSL, (i + 1) * TSL)
                for cps, rows in ((ctxA, slice(0, 64)), (ctxB, slice(64, 128))):
                    # custom-DVE ops misread PSUM on hw: bounce rowsum via SBUF
                    rs = rinv_pool.tile([1, TSL], f32, tag="rsum", name="rsum")
                    nc.vector.tensor_copy(rs[:], cps[DK : DK + 1, :])
                    r = rinv_pool.tile([1, TSL], f32, tag="rinv", name="rinv")
                    nc.vector.reciprocal_approx_fast(r[:], rs[:])
                    rbc = rbc_pool.tile([DK, TSL], f32, tag="rbc", name="rbc")
                    nc.gpsimd.partition_broadcast(rbc[:], r[:])
                    nc.vector.tensor_mul(ctxT_sb[p][rows, isl], cps[0:DK, :], rbc[:])

            # ---- main i-outer loop ----
            # projections for i+1 are emitted between attn(i) and out(i) so
            # the PE has independent work while the last pair normalizes.
            for ot in range(n_qk):
                qk_proj(ot, 0)
            for tt in range(0, JPI):
                v_proj(tt)
            for i in range(n_it):
                # pairs 0-1 of iteration i were already emitted at the end
                # of iteration i-1 (pulled ahead so ScalarE gets exp work
                # during the projection segment).
                for p in range(0 if i == 0 else 2, n_pairs):
                    attn_pair(p, i)
                if i + 1 < n_it:
                    qk_proj(0, i + 1)
                    qk_proj(1, i + 1)
                    for tt in range(JPI * (i + 1), JPI * (i + 2)):
                        v_proj(tt)
                    attn_pair(0, i + 1)
                    qk_proj(2, i + 1)
                    qk_proj(3, i + 1)
                    attn_pair(1, i + 1)
                    for ot in range(4, n_qk):
                        qk_proj(ot, i + 1)
                for tt in range(JPI * i, JPI * (i + 1)):
                    for oh in range(n_oh):
                        out_proj(tt, oh)

    nc.compile()
    return nc


def make_mask_square(ts=TS):
    """[128, ts] strict lower-triangular: cell (s, t) = MASK_VAL iff s > t."""
    s = np.arange(128)[:, None]
    t = np.arange(ts)[None, :]
    return np.where(s > t, MASK_VAL, 0.0).astype(np.float32)


def make_core_inputs(x_b, W_qkv, b_qkv, W_out, heads, C_sz=C, T_sz=T):
    """Build the per-core input map (numpy, host-side)."""
    n_pairs = len(heads) // 2
    n_qk = 2 * n_pairs
    VW = len(heads) * DK
    xT = np.ascontiguousarray(x_b.T).astype(BF16)
    wqk = np.empty((C_sz, n_qk * 128), np.float32)
    bqk = np.empty((128, n_qk), np.float32)
    wv = np.empty((C_sz, VW), np.float32)
    bv = np.empty((1, VW), np.float32)
    wo = np.empty((n_pairs * 128, C_sz), np.float32)
    for p in range(n_pairs):
        hA, hB = heads[2 * p], heads[2 * p + 1]
        # q tile (scaled by 1/sqrt(dk)=1/8), k tile
        for half, h in ((0, hA), (1, hB)):
            r0 = h * 3 * DK
            wqk[:, 2 * p * 128 + half * 64 : 2 * p * 128 + half * 64 + 64] = (
                W_qkv[r0 : r0 + DK].T / math.sqrt(DK)
            )
            bqk[half * 64 : half * 64 + 64, 2 * p] = b_qkv[r0 : r0 + DK] / math.sqrt(DK)
            wqk[:, (2 * p + 1) * 128 + half * 64 : (2 * p + 1) * 128 + half * 64 + 64] = (
                W_qkv[r0 + DK : r0 + 2 * DK].T
            )
            bqk[half * 64 : half * 64 + 64, 2 * p + 1] = b_qkv[r0 + DK : r0 + 2 * DK]
            wo[p * 128 + half * 64 : p * 128 + half * 64 + 64, :] = W_out[
                :, h * DK : (h + 1) * DK
            ].T
    for hh, h in enumerate(heads):
        r0 = h * 3 * DK + 2 * DK
        wv[:, hh * DK : (hh + 1) * DK] = W_qkv[r0 : r0 + DK].T
        bv[0, hh * DK : (hh + 1) * DK] = b_qkv[r0 : r0 + DK]
    return {
        "xT": xT,
        "wqkT": wqk.astype(BF16),
        "wvT": wv.astype(BF16),
        "bqk": bqk.astype(np.float32),
        "bv": bv.astype(BF16),
        "woT": wo.astype(BF16),
        "masksq": make_mask_square().astype(BF16),
        "ident": np.eye(128, dtype=np.float32).astype(BF16),
    }


_NC_CACHE = {}


def kernel(x, W_qkv, b_qkv, W_out, b_out, _trace=False):
    x = np.asarray(x, dtype=np.float32)
    W_qkv = np.asarray(W_qkv, dtype=np.float32)
    b_qkv = np.asarray(b_qkv, dtype=np.float32)
    W_out = np.asarray(W_out, dtype=np.float32)
    b_out = np.asarray(b_out, dtype=np.float32)

    from concourse.bass_utils import run_bass_kernel_spmd

    key = ("full", C, T, 4)
    if key not in _NC_CACHE:
        _NC_CACHE[key] = build_program(C, T, n_pairs=4, num_devices=1)
    nc = _NC_CACHE[key]

    in_maps = []
    for core in range(NCORES):
        b, hg = divmod(core, 2)
        heads = list(range(hg * 8, hg * 8 + 8))
        in_maps.append(make_core_inputs(x[b], W_qkv, b_qkv, W_out, heads))

    res = run_bass_kernel_spmd(nc, in_maps, list(range(NCORES)), trace=_trace)
    kernel._last_results = res

    out = np.broadcast_to(b_out, (B, T, C)).astype(np.float32).copy()
    for core in range(NCORES):
        b = core // 2
        out[b] += res.results[core]["out"]
    return out


# revision 21
# speedup vs baseline: 1.0684x; 1.0145x over previous
"""Causal self-attention Trainium2 kernel (B=4, T=2048, D=1024, H=16).

Sharding: 8 cores = 4 batches x 2 head-groups (8 heads each). Each core
computes its batch's qkv projection restricted to its 8 heads, causal
attention for those heads, and a partial out-projection over its 512 ctx
channels. Host sums the two partials per batch and adds b_out.

Per-core layout choices (all matmuls bf16 with fp32 PSUM accumulation):
  - xT [C, T]: channels on partitions (contraction dim for projections).
  - qkT: per head-pair p, a q-tile [128, T] (head A rows 0:64, head B rows
    64:128) and a k-tile [128, T]. Produced directly transposed by making
    W the stationary operand. The 1/sqrt(dk) scale is folded into Wq/bq.
  - scoresT[s, t] blocks [128, 512]: lhsT=kT (K=64 rows), rhs=qT. Heads A/B
    are row-packed (tile_position rows 0:64 / 64:128) and run concurrently.
    Diagonal blocks only compute the causally needed t-range.
  - causal mask: diagonal 128x128 squares get an extra K=128 identity
    matmul accumulating a {0, -30000} triangular pattern; exp() gives 0.
  - softmax: no max-subtraction (scores are within +-10 by construction),
    exp on ScalarE PSUM->SBUF bf16.
  - ctx: v stored naturally [s, d] with a ones column appended per head
    (v_ext [128, 8*65]); lhsT=v_ext (M=65) so PSUM row 64 accumulates the
    softmax denominator. Normalize = reciprocal_approx_fast + gpsimd
    partition_broadcast + DVE mul into the bf16 ctxT copy.
  - out projection: ctxT pair-tiles [128, T] are the stationary operand
    against W_outT; b_out is added on the host (once per batch).

The main loop is i-tile-outer (t blocks of 512) so qk/v projection work,
attention for all 4 pairs, and the out-projection interleave: the PE
stays dense (HAM stays at K=8/8) and ScalarE exp overlaps matmuls.
"""

import math

import numpy as np
import ml_dtypes

B, T, C = 4, 2048, 1024
H, DK = 16, 64
NCORES = 8
TS = 128  # s-tile (partition granularity)
TSL = 512  # t free-dim tile (one PSUM bank of fp32)
MASK_VAL = -30000.0
BF16 = ml_dtypes.bfloat16


def build_program(C_sz=C, T_sz=T, n_pairs=4, num_devices=1):
    import concourse.mybir as mybir
    from concourse import bacc
    from concourse.tile import TileContext

    dt = mybir.dt
    f32 = dt.float32
    bf16 = dt.bfloat16
    AF = mybir.ActivationFunctionType

    n_ct = C_sz // 128  # contraction tiles for projections
    n_qk = 2 * n_pairs  # qk o-tiles (128 channels each)
    VW = n_pairs * 2 * DK  # v channels (natural order)
    n_tt = T_sz // TS
    n_it = T_sz // TSL
    JPI = TSL // TS  # s-tiles per i-tile (4)
    OW = min(TSL, C_sz)  # output column tile width
    n_oh = C_sz // OW  # output column halves
    VEW = n_pairs * 2 * (DK + 1)  # v_ext width (65 per head)

    nc = bacc.Bacc(
        "TRN2",
        target_bir_lowering=False,
        debug=False,
        num_devices=num_devices,
    )

    xT_d = nc.dram_tensor("xT", [C_sz, T_sz], bf16, kind="ExternalInput").ap()
    wqk_d = nc.dram_tensor("wqkT", [C_sz, n_qk * 128], bf16, kind="ExternalInput").ap()
    wv_d = nc.dram_tensor("wvT", [C_sz, VW], bf16, kind="ExternalInput").ap()
    bqk_d = nc.dram_tensor("bqk", [128, n_qk], f32, kind="ExternalInput").ap()
    bv_d = nc.dram_tensor("bv", [1, VW], bf16, kind="ExternalInput").ap()
    wo_d = nc.dram_tensor("woT", [n_pairs * 128, C_sz], bf16, kind="ExternalInput").ap()
    mask_d = nc.dram_tensor("masksq", [128, TS], bf16, kind="ExternalInput").ap()
    id_d = nc.dram_tensor("ident", [128, 128], bf16, kind="ExternalInput").ap()
    out_d = nc.dram_tensor("out", [T_sz, C_sz], f32, kind="ExternalOutput").ap()

    with TileContext(nc) as tc:
        with (
            tc.tile_pool(name="const", bufs=1) as const_pool,
            tc.tile_pool(name="big", bufs=1) as big_pool,
            tc.tile_pool(name="attn", bufs=10) as attn_pool,
            tc.tile_pool(name="rinv", bufs=6) as rinv_pool,
            tc.tile_pool(name="rbc", bufs=6) as rbc_pool,
            tc.tile_pool(name="outsb", bufs=6) as outsb_pool,
            tc.tile_pool(name="sc", bufs=2, space="PSUM") as sc_ps,
            tc.tile_pool(name="mm", bufs=4, space="PSUM") as mm_ps,
        ):
            # ---- weight/activation loads (first compute inputs first) ----
            xT_sb = []
            wqk_sb = []
            wv_sb = []
            for ci in range(n_ct):
                t = big_pool.tile([128, T_sz], bf16, tag=f"xT{ci}", name=f"xT{ci}")
                nc.sync.dma_start(t[:], xT_d[ci * 128 : (ci + 1) * 128, :])
                xT_sb.append(t)
                t = big_pool.tile(
                    [128, n_qk * 128], bf16, tag=f"wqk{ci}", name=f"wqk{ci}"
                )
                nc.sync.dma_start(t[:], wqk_d[ci * 128 : (ci + 1) * 128, :])
                wqk_sb.append(t)
            bqk_sb = const_pool.tile([128, n_qk], f32, tag="bqk", name="bqk")
            nc.sync.dma_start(bqk_sb[:], bqk_d)
            for ci in range(n_ct):
                t = big_pool.tile([128, VW], bf16, tag=f"wv{ci}", name=f"wv{ci}")
                nc.sync.dma_start(t[:], wv_d[ci * 128 : (ci + 1) * 128, :])
                wv_sb.append(t)
            bv_sb = const_pool.tile([1, VW], bf16, tag="bv", name="bv")
            nc.sync.dma_start(bv_sb[:], bv_d)
            bv_bc = const_pool.tile([128, VW], bf16, tag="bv_bc", name="bv_bc")
            nc.gpsimd.partition_broadcast(bv_bc[:], bv_sb[:])
            ident_sb = const_pool.tile([128, 128], bf16, tag="ident", name="ident")
            nc.sync.dma_start(ident_sb[:], id_d)
            mask_sb = const_pool.tile([128, TS], bf16, tag="mask", name="mask")
            nc.sync.dma_start(mask_sb[:], mask_d)
            wo_sb = []
            for p in range(n_pairs):
                t = big_pool.tile([128, C_sz], bf16, tag=f"wo{p}", name=f"wo{p}")
                nc.sync.dma_start(t[:], wo_d[p * 128 : (p + 1) * 128, :])
                wo_sb.append(t)

            qkT_sb = [
                big_pool.tile([128, T_sz], bf16, tag=f"qkT{ot}", name=f"qkT{ot}")
                for ot in range(n_qk)
            ]
            vext_sb = [
                big_pool.tile([128, VEW], bf16, tag=f"vext{tt}", name=f"vext{tt}")
                for tt in range(n_tt)
            ]
            ctxT_sb = [
                big_pool.tile([128, T_sz], bf16, tag=f"ctxT{p}", name=f"ctxT{p}")
                for p in range(n_pairs)
            ]

            def qk_proj(ot, i):
                ps = mm_ps.tile([128, TSL], f32, tag="mm", name="mm")
                for ci in range(n_ct):
                    nc.tensor.matmul(
                        ps[:],
                        lhsT=wqk_sb[ci][:, ot * 128 : (ot + 1) * 128],
                        rhs=xT_sb[ci][:, i * TSL : (i + 1) * TSL],
                        start=(ci == 0),
                        stop=(ci == n_ct - 1),
                    )
                nc.scalar.activation(
                    qkT_sb[ot][:, i * TSL : (i + 1) * TSL],
                    ps[:],
                    AF.Identity,
                    bias=bqk_sb[:, ot : ot + 1],
                )

            def v_proj(tt):
                ps = mm_ps.tile([128, VW], f32, tag="mm", name="mm")
                for ci in range(n_ct):
                    nc.tensor.matmul(
                        ps[:],
                        lhsT=xT_sb[ci][:, tt * TS : (tt + 1) * TS],
                        rhs=wv_sb[ci][:],
                        start=(ci == 0),
                        stop=(ci == n_ct - 1),
                    )
                vx = vext_sb[tt]
                vx3 = vx[:].rearrange("p (h e) -> p h e", e=DK + 1)
                nc.gpsimd.memset(vx3[:, :, DK : DK + 1], 1.0)
                nc.vector.scalar_tensor_tensor(
                    vx3[:, :, 0:DK],
                    ps[:].rearrange("p (h e) -> p h e", e=DK),
                    1.0,
                    bv_bc[:].rearrange("p (h e) -> p h e", e=DK),
                    op0=mybir.AluOpType.mult,
                    op1=mybir.AluOpType.add,
                )

            def out_proj(tt, oh):
                ps = mm_ps.tile([128, OW], f32, tag="mm", name="mm")
                for p in range(n_pairs):
                    nc.tensor.matmul(
                        ps[:],
                        lhsT=ctxT_sb[p][:, tt * TS : (tt + 1) * TS],
                        rhs=wo_sb[p][:, oh * OW : (oh + 1) * OW],
                        start=(p == 0),
                        stop=(p == n_pairs - 1),
                    )
                ob = outsb_pool.tile([128, OW], f32, tag="outsb", name="outsb")
                nc.scalar.activation(ob[:], ps[:], AF.Identity)
                nc.sync.dma_start(
                    out_d[tt * TS : (tt + 1) * TS, oh * OW : (oh + 1) * OW],
                    ob[:],
                )

            def attn_pair(p, i):
                qt, kt = qkT_sb[2 * p], qkT_sb[2 * p + 1]
                nj = JPI * (i + 1)
                ctxA = mm_ps.tile([DK + 1, TSL], f32, tag="mm", name="mm")
                ctxB = mm_ps.tile([DK + 1, TSL], f32, tag="mm", name="mm")
                for j in range(nj):
                    diag = j >= JPI * i
                    pi = j - JPI * i if diag else 0
                    t0 = pi * TS  # first causally-live t column in this block
                    ps = sc_ps.tile([128, 2 * TSL], f32, tag="sc", name="sc")
                    nc.tensor.matmul(
                        ps[:, t0:TSL],
                        lhsT=kt[0:64, j * TS : (j + 1) * TS],
                        rhs=qt[0:64, i * TSL + t0 : (i + 1) * TSL],
                        start=True,
                        stop=not diag,
                        skip_group_check=True,
                    )
                    nc.tensor.matmul(
                        ps[:, TSL + t0 : 2 * TSL],
                        lhsT=kt[64:128, j * TS : (j + 1) * TS],
                        rhs=qt[64:128, i * TSL + t0 : (i + 1) * TSL],
                        start=True,
                        stop=not diag,
                        skip_group_check=True,
                    )
                    if diag:
                        nc.tensor.matmul(
                            ps[:, t0 : t0 + TS],
                            lhsT=ident_sb[:],
                            rhs=mask_sb[:],
                            start=False,
                            stop=True,
                            skip_group_check=True,
                        )
                        nc.tensor.matmul(
                            ps[:, TSL + t0 : TSL + t0 + TS],
                            lhsT=ident_sb[:],
                            rhs=mask_sb[:],
                            start=False,
                            stop=True,
                            skip_group_check=True,
                        )
                    a = attn_pool.tile([128, 2 * TSL], bf16, tag="attn", name="attn")
                    a3 = a[:].rearrange("p (c w) -> p c w", c=2)
                    ps3 = ps[:].rearrange("p (c w) -> p c w", c=2)
                    nc.scalar.activation(a3[:, :, t0:TSL], ps3[:, :, t0:TSL], AF.Exp)
                    nc.tensor.matmul(
                        ctxA[:, t0:TSL],
                        lhsT=vext_sb[j][:, (2 * p) * (DK + 1) : (2 * p + 1) * (DK + 1)],
                        rhs=a[:, t0:TSL],
                        start=(j == 0),
                        stop=(j == nj - 1),
                    )
                    nc.tensor.matmul(
                        ctxB[:, t0:TSL],
                        lhsT=vext_sb[j][
                            :, (2 * p + 1) * (DK + 1) : (2 * p + 2) * (DK + 1)
                        ],
                        rhs=a[:, TSL + t0 : 2 * TSL],
                        start=(j == 0),
                        stop=(j == nj - 1),
                    )
                isl = slice(i * TSL, (i + 1) * TSL)
                for cps, rows in ((ctxA, slice(0, 64)), (ctxB, slice(64, 128))):
                    # custom-DVE ops misread PSUM on hw: bounce rowsum via SBUF
                    rs = rinv_pool.tile([1, TSL], f32, tag="rsum", name="rsum")
                    nc.vector.tensor_copy(rs[:], cps[DK : DK + 1, :])
                    r = rinv_pool.tile([1, TSL], f32, tag="rinv", name="rinv")
                    nc.vector.reciprocal_approx_fast(r[:], rs[:])
                    rbc = rbc_pool.tile([DK, TSL], f32, tag="rbc", name="rbc")
                    nc.gpsimd.partition_broadcast(rbc[:], r[:])
                    nc.vector.tensor_mul(ctxT_sb[p][rows, isl], cps[0:DK, :], rbc[:])

            # ---- main i-outer loop ----
            # projections for i+1 are emitted between attn(i) and out(i) so
            # the PE has independent work while the last pair normalizes.
            for ot in range(n_qk):
                qk_proj(ot, 0)
            for tt in range(0, JPI):
                v_proj(tt)
            for i in range(n_it):
                # pairs 0-2 of iteration i were already emitted at the end
                # of iteration i-1 (pulled ahead so ScalarE gets exp work
                # during the projection segment).
                for p in range(0 if i == 0 else 3, n_pairs):
                    attn_pair(p, i)
                if i + 1 < n_it:
                    qk_proj(0, i + 1)
                    qk_proj(1, i + 1)
                    for tt in range(JPI * (i + 1), JPI * (i + 2)):
                        v_proj(tt)
                    attn_pair(0, i + 1)
                    qk_proj(2, i + 1)
                    qk_proj(3, i + 1)
                    attn_pair(1, i + 1)
                    qk_proj(4, i + 1)
                    qk_proj(5, i + 1)
                    attn_pair(2, i + 1)
                    qk_proj(6, i + 1)
                    qk_proj(7, i + 1)
                for tt in range(JPI * i, JPI * (i + 1)):
                    for oh in range(n_oh):
                        out_proj(tt, oh)

    nc.compile()
    return nc


def make_mask_square(ts=TS):
    """[128, ts] strict lower-triangular: cell (s, t) = MASK_VAL iff s > t."""
    s = np.arange(128)[:, None]
    t = np.arange(ts)[None, :]
    return np.where(s > t, MASK_VAL, 0.0).astype(np.float32)


def make_core_inputs(x_b, W_qkv, b_qkv, W_out, heads, C_sz=C, T_sz=T):
    """Build the per-core input map (numpy, host-side)."""
    n_pairs = len(heads) // 2
    n_qk = 2 * n_pairs
    VW = len(heads) * DK
    xT = np.ascontiguousarray(x_b.T).astype(BF16)
    wqk = np.empty((C_sz, n_qk * 128), np.float32)
    bqk = np.empty((128, n_qk), np.float32)
    wv = np.empty((C_sz, VW), np.float32)
    bv = np.empty((1, VW), np.float32)
    wo = np.empty((n_pairs * 128, C_sz), np.float32)
    for p in range(n_pairs):
        hA, hB = heads[2 * p], heads[2 * p + 1]
        # q tile (scaled by 1/sqrt(dk)=1/8), k tile
        for half, h in ((0, hA), (1, hB)):
            r0 = h * 3 * DK
            wqk[:, 2 * p * 128 + half * 64 : 2 * p * 128 + half * 64 + 64] = (
                W_qkv[r0 : r0 + DK].T / math.sqrt(DK)
            )
            bqk[half * 64 : half * 64 + 64, 2 * p] = b_qkv[r0 : r0 + DK] / math.sqrt(DK)
            wqk[:, (2 * p + 1) * 128 + half * 64 : (2 * p + 1) * 128 + half * 64 + 64] = (
                W_qkv[r0 + DK : r0 + 2 * DK].T
            )
            bqk[half * 64 : half * 64 + 64, 2 * p + 1] = b_qkv[r0 + DK : r0 + 2 * DK]
            wo[p * 128 + half * 64 : p * 128 + half * 64 + 64, :] = W_out[
                :, h * DK : (h + 1) * DK
            ].T
    for hh, h in enumerate(heads):
        r0 = h * 3 * DK + 2 * DK
        wv[:, hh * DK : (hh + 1) * DK] = W_qkv[r0 : r0 + DK].T
        bv[0, hh * DK : (hh + 1) * DK] = b_qkv[r0 : r0 + DK]
    return {
        "xT": xT,
        "wqkT": wqk.astype(BF16),
        "wvT": wv.astype(BF16),
        "bqk": bqk.astype(np.float32),
        "bv": bv.astype(BF16),
        "woT": wo.astype(BF16),
        "masksq": make_mask_square().astype(BF16),
        "ident": np.eye(128, dtype=np.float32).astype(BF16),
    }


_NC_CACHE = {}


def kernel(x, W_qkv, b_qkv, W_out, b_out, _trace=False):
    x = np.asarray(x, dtype=np.float32)
    W_qkv = np.asarray(W_qkv, dtype=np.float32)
    b_qkv = np.asarray(b_qkv, dtype=np.float32)
    W_out = np.asarray(W_out, dtype=np.float32)
    b_out = np.asarray(b_out, dtype=np.float32)

    from concourse.bass_utils import run_bass_kernel_spmd

    key = ("full", C, T, 4)
    if key not in _NC_CACHE:
        _NC_CACHE[key] = build_program(C, T, n_pairs=4, num_devices=1)
    nc = _NC_CACHE[key]

    in_maps = []
    for core in range(NCORES):
        b, hg = divmod(core, 2)
        heads = list(range(hg * 8, hg * 8 + 8))
        in_maps.append(make_core_inputs(x[b], W_qkv, b_qkv, W_out, heads))

    res = run_bass_kernel_spmd(nc, in_maps, list(range(NCORES)), trace=_trace)
    kernel._last_results = res

    out = np.broadcast_to(b_out, (B, T, C)).astype(np.float32).copy()
    for core in range(NCORES):
        b = core // 2
        out[b] += res.results[core]["out"]
    return out
